# revision 9
# baseline (speedup 1.0000x reference)
"""Block-diagonal Hamming-similarity gram kernel for TRN2 (8 NeuronCores).

Problem: x [B=4, N=1024, L=512, A=21] fp32; 16 consecutive groups of 64
sequences per batch; per group compute sims = (Xg @ Xg.T) / L where Xg is
[64, L*A=10752]; output [B, N, N] is NaN-filled with the 64x64 blocks
written on the block diagonal.

Strategy:
- Shard the 64 independent (batch, group) blocks across 8 cores (8 each).
- Host marshals each block to a feature-major fp16 layout
  xt[t, p, c*64+n] = x[b, g*64+n, c*128+p], so every 128-row contraction
  chunk is a ready-to-use [K=128, 64] SBUF tile for both matmul operands
  (lhsT == rhs -> gram), with fully contiguous per-partition DMA.
- fp16 runs the PE at 1 cycle/row (fp32 is 4) and halves DMA bytes;
  measured accuracy vs the fp32 reference is ~2.4e-5 relative.
- Per block: accumulate 84 chunk matmuls into one PSUM [64, 64], scale by
  1/L during the PSUM->SBUF copy, DMA out. NaN canvas + block scatter on
  host.
"""

import numpy as np

B, N, L, A = 4, 1024, 512, 21
G, GS = 16, 64
LA = L * A           # 10752
K = 128              # contraction tile (partition dim)
C = LA // K          # 84 chunks
NBLK = B * G         # 64 independent gram blocks
NCORES = 8
BPC = NBLK // NCORES # 8 blocks per core

_CACHE = {}


def _build_nc():
    if "nc" in _CACHE:
        return _CACHE["nc"]
    import concourse.mybir as mybir
    from concourse import bacc
    from concourse.tile import TileContext

    NDMA = 4             # input DMAs; 5 HWDGE DMAs total keeps every DMA on
    BPD = BPC // NDMA    # its own sem lane and the tail drain under the
                         # per-instruction sync-wait limit.

    # Bacc (not raw Bass): its finalize() legalizes multi-sem waits into
    # event-semaphore trees, which the TRN2 ISA requires (1 wait/inst).
    nc = bacc.Bacc("TRN2", target_bir_lowering=False)
    xt = nc.declare_dram_parameter("xt", [BPC, K, C * GS], mybir.dt.float16, isOutput=False)
    sims = nc.declare_dram_parameter("sims", [GS, BPC * GS], mybir.dt.float32, isOutput=True)

    with TileContext(nc) as tc:
        with (
            tc.tile_pool(name="xin", bufs=NDMA) as xpool,
            tc.tile_pool(name="ps", bufs=4, space="PSUM") as ppool,
            tc.tile_pool(name="osb", bufs=1) as opool,
        ):
            xtiles = []
            for d in range(NDMA):
                xtile = xpool.tile([K, BPD * C * GS], mybir.dt.float16, tag="x")
                src = xt[d * BPD:(d + 1) * BPD].rearrange("g p f -> p g f")
                dst = xtile.rearrange("p (g f) -> p g f", g=BPD)
                nc.sync.dma_start(out=dst, in_=src)
                xtiles.append(xtile)
            ob = opool.tile([GS, BPC * GS], mybir.dt.float32, tag="o")
            for g in range(BPC):
                d, j = divmod(g, BPD)
                base = j * C * GS
                ps = ppool.tile([GS, GS], mybir.dt.float32, tag="ps")
                for c in range(C):
                    sl = xtiles[d][:, base + c * GS: base + (c + 1) * GS]
                    nc.tensor.matmul(ps, lhsT=sl, rhs=sl, start=(c == 0), stop=(c == C - 1))
                nc.scalar.mul(ob[:, g * GS:(g + 1) * GS], ps, 1.0 / L)
            nc.sync.dma_start(out=sims[:], in_=ob)

    nc.finalize()
    _CACHE["nc"] = nc
    return nc


def _marshal(x):
    # [B,N,L,A] -> blocks [NBLK, K, C*GS] fp16, feature-major per block.
    xs = np.asarray(x, dtype=np.float32).reshape(B, G, GS, C, K)
    xt = xs.transpose(0, 1, 4, 3, 2)                 # [b, g, p, c, n]
    return np.ascontiguousarray(xt, dtype=np.float16).reshape(NBLK, K, C * GS)


def run(x, trace=False):
    from concourse.bass_utils import run_bass_kernel_spmd

    nc = _build_nc()
    xt16 = _marshal(x)
    in_maps = [{"xt": xt16[m * BPC:(m + 1) * BPC]} for m in range(NCORES)]
    res = run_bass_kernel_spmd(nc, in_maps, list(range(NCORES)), trace=trace)
    # per-core result is [GS, BPC*GS] = [i, (g j)] -> [g, i, j]
    sims = np.stack(
        [
            res.results[m]["sims"].reshape(GS, BPC, GS).transpose(1, 0, 2)
            for m in range(NCORES)
        ],
    ).reshape(B, G, GS, GS)

    out = np.full((B, N, N), np.nan, dtype=np.float32)
    for g in range(G):
        out[:, g * GS:(g + 1) * GS, g * GS:(g + 1) * GS] = sims[:, g]
    return out, res


def kernel(x):
    out, _ = run(x, trace=False)
    return out
